# revision 4
# baseline (speedup 1.0000x reference)
"""Multi-head causal attention on 8 Trainium2 NeuronCores.

Sharding: core c -> batch b = c // 4, head group g = c % 4 (4 of 16 heads).
Each core computes q/k/v for its 4 heads, causal softmax attention, and a
partial output  z_norm @ W_O[heads]  of shape [S, D].  Host sums the 4
head-group partials per batch and adds b_O (+ the exact b_V correction
sum_h b_V[h] @ W_O[h], since softmax rows sum to 1).

v2 design (vs v1): fp16 datapath end to end (PE full rate at any free dim,
FWL weight loads, half DMA traffic), everything SBUF-resident (no DRAM
scratch round trip), V computed directly in natural [s, e] layout in the
projection phase (x.T slices as the stationary operand) so the per-head V
transposes disappear, and the emission order software-pipelines phases:
the q/k/v projection groups and output-projection groups are interleaved
into the attention phase as PE filler work so the PE never idles while the
scalar engine runs exp().

Per-core phases (PSUM budget: scores 2x[128,2,512] + proj 1x[128,512] +
z 2x[128,129] + transpose 1x[128,128] = 8 banks):
  A: qT/kT[h] = W.T @ x.T (+bias, q pre-scaled by 1/sqrt(dh)),
     v_nat[jt] = x.T[jt].T @ W_V  (all heads at once, free dim 512).
  B: per head: scoresT[j,i] = kT.T @ qT in 2-j-tile waves, additive causal
     mask on the diagonal 128-block, one exp() per wave on ACT, PV with a
     fused ones-column on v (row sums land in column E of the z psum),
     per-partition normalization, PE transpose into zT.
  C: out[s_tile, d_chunk] = sum_h zT_h.T @ Wo_h, staged fp16, DMA out.
"""

import sys

for _p in ("/opt/trn_rl_repo",):
    if _p not in sys.path:
        sys.path.insert(0, _p)

import numpy as np

import concourse.bass as bass
from concourse import bacc
import concourse.mybir as mybir
import concourse.tile as tile
from concourse.bass_utils import run_bass_kernel_spmd
from concourse.masks import make_identity

F32 = mybir.dt.float32
F16 = mybir.dt.float16

B, S, D, H, E = 2, 2048, 2048, 16, 128
HL = 4          # heads per core
NCORES = 8
P = 128         # partitions
CH = 512        # free-dim chunk
S_T = S // P    # 16 seq tiles
S_C = S // CH   # 4 seq chunks
D_T = D // P    # 16 model-dim subtiles
D_C = D // CH   # 4 model-dim chunks
INV_SQRT_E = 1.0 / float(np.sqrt(E))
WAVE = 2        # j-tiles per scores wave (one PSUM pair-tile)


def _trace_kernel(tc, xt, wq, wk, wv, wo, bq, bk, outp):
    nc = tc.nc
    ts = bass.ts

    xt3 = xt.rearrange("(o p) s -> p o s", p=P)            # [128, 16, 2048]
    wq3 = wq.rearrange("(o p) e -> p o e", p=P)            # [128, 16, 512]
    wk3 = wk.rearrange("(o p) e -> p o e", p=P)
    wv3 = wv.rearrange("(o p) e -> p o e", p=P)
    wo3 = wo.rearrange("(h p) d -> p h d", p=P)            # [128, 4, 2048]
    out3 = outp.rearrange("(t p) d -> t p d", p=P)         # [16, 128, 2048]

    from contextlib import ExitStack

    with ExitStack() as top:
        const_pool = top.enter_context(tc.tile_pool(name="consts", bufs=1))
        xpool = top.enter_context(tc.tile_pool(name="x", bufs=1))
        wpool = top.enter_context(tc.tile_pool(name="w", bufs=1))
        wopool = top.enter_context(tc.tile_pool(name="wo", bufs=1))
        qkpool = top.enter_context(tc.tile_pool(name="qk", bufs=2))
        vpool = top.enter_context(tc.tile_pool(name="v", bufs=1))
        ztpool = top.enter_context(tc.tile_pool(name="zt", bufs=1))
        expp_s = top.enter_context(tc.tile_pool(name="exps", bufs=1))
        expp_b = top.enter_context(tc.tile_pool(name="expb", bufs=1))
        zsb = top.enter_context(tc.tile_pool(name="zsb", bufs=2))
        recp = top.enter_context(tc.tile_pool(name="rec", bufs=2))
        ostage = top.enter_context(tc.tile_pool(name="ost", bufs=3))
        psA = top.enter_context(tc.tile_pool(name="psA", bufs=1, space="PSUM"))
        psS = top.enter_context(tc.tile_pool(name="psS", bufs=2, space="PSUM"))
        psZ = top.enter_context(tc.tile_pool(name="psZ", bufs=2, space="PSUM"))
        psT = top.enter_context(tc.tile_pool(name="psT", bufs=1, space="PSUM"))

        # ---------------- constants ----------------
        identity_f = const_pool.tile([P, P], F32)
        make_identity(nc, identity_f)
        identity = const_pool.tile([P, P], F16)
        nc.vector.tensor_copy(identity, identity_f)

        # transposed causal triangle for the diagonal 128-block of scoresT:
        # valid iff local col >= p  (j <= i)
        dmask = const_pool.tile([P, P], F32)
        nc.gpsimd.memset(dmask, 0.0)
        nc.gpsimd.affine_select(
            out=dmask,
            in_=dmask,
            compare_op=mybir.AluOpType.is_ge,
            fill=-30000.0,
            base=0,
            pattern=[[1, P]],
            channel_multiplier=-1,
        )

        biases = const_pool.tile([P, 2, HL], F32)
        nc.gpsimd.dma_start(biases[:, 0, :], bq.rearrange("(h p) -> p h", p=P))
        nc.gpsimd.dma_start(biases[:, 1, :], bk.rearrange("(h p) -> p h", p=P))

        # ---------------- input DMAs ----------------
        x_sb = xpool.tile([P, D_T, S], F16)
        wv_sb = wpool.tile([P, D_T, HL * E], F16, name="wv")
        wq_sb = wpool.tile([P, D_T, HL * E], F16, name="wq")
        wk_sb = wpool.tile([P, D_T, HL * E], F16, name="wk")
        wo_sb = wopool.tile([P, HL, D], F16)

        for d in range(D_T):
            nc.sync.dma_start(wv_sb[:, d, :], wv3[:, d, :])
        for d in range(D_T):
            nc.sync.dma_start(x_sb[:, d, ts(0, CH)], xt3[:, d, ts(0, CH)])
        for d in range(D_T):
            nc.sync.dma_start(wq_sb[:, d, :], wq3[:, d, :])
        for d in range(D_T):
            nc.sync.dma_start(wk_sb[:, d, :], wk3[:, d, :])
        for c in range(1, S_C):
            for d in range(D_T):
                nc.sync.dma_start(x_sb[:, d, ts(c, CH)], xt3[:, d, ts(c, CH)])
        for lh in range(HL):
            for dc in range(D_C):
                nc.sync.dma_start(wo_sb[:, lh, ts(dc, CH)], wo3[:, lh, ts(dc, CH)])

        # v natural layout [j_in_tile, jt, h, e + ones-column], fp16
        v_nat = vpool.tile([P, S_T, HL, E + 1], F16)
        for jt in range(S_T):
            nc.vector.memset(v_nat[:, jt, :, E : E + 1], 1.0)

        zT = ztpool.tile([P, HL, S], F16)

        qT = {}
        kT = {}

        # ---------------- emission helpers ----------------
        def a_group(m, h, c):
            """qT/kT projection group: one head, one 512-seq chunk."""
            dst_map = qT if m == 0 else kT
            if c == 0:
                dst_map[h] = qkpool.tile([P, S], F16, name="qT" if m == 0 else "kT")
            w_sb = wq_sb if m == 0 else wk_sb
            ps = psA.tile([P, CH], F32, name="psA")
            for d in range(D_T):
                nc.tensor.matmul(
                    ps,
                    w_sb[:, d, ts(h, E)],
                    x_sb[:, d, ts(c, CH)],
                    start=(d == 0),
                    stop=(d == D_T - 1),
                )
            # q: bq is pre-scaled by 1/sqrt(E) on host
            nc.vector.tensor_scalar(
                dst_map[h][:, ts(c, CH)], ps,
                INV_SQRT_E if m == 0 else 1.0,
                biases[:, m, h, None],
                op0=mybir.AluOpType.mult,
                op1=mybir.AluOpType.add,
            )

        def v_group(jt):
            """v_nat[jt] for all 4 heads at once (free dim 512)."""
            ps = psA.tile([P, CH], F32, name="psA")
            for d in range(D_T):
                nc.tensor.matmul(
                    ps,
                    x_sb[:, d, ts(jt, P)],
                    wv_sb[:, d, :],
                    start=(d == 0),
                    stop=(d == D_T - 1),
                )
            for lh in range(HL):
                nc.vector.tensor_copy(
                    v_nat[:, jt, lh, :E], ps[:, ts(lh, E)]
                )

        def c_group(t, dc):
            """Output projection for one (seq-tile, d-chunk)."""
            ps = psS.tile([P, CH], F32, name="sps")
            for lh in range(HL):
                nc.tensor.matmul(
                    ps,
                    zT[:, lh, ts(t, P)],
                    wo_sb[:, lh, ts(dc, CH)],
                    start=(lh == 0),
                    stop=(lh == HL - 1),
                )
            ot = ostage.tile([P, CH], F16, name="ot")
            nc.vector.tensor_copy(ot, ps)
            nc.sync.dma_start(out3[t, :, ts(dc, CH)], ot)

        def b_head(h, fillers):
            """Causal attention for one head; pops filler emissions to keep
            the PE busy while ACT runs exp()."""
            def fill(n=1):
                for _ in range(n):
                    if fillers:
                        fillers.pop(0)()

            for c in range(S_C):
                n_jt = S_C * c + 4
                expT = (expp_s if c < 2 else expp_b).tile(
                    [P, 8 if c < 2 else 16, CH], F16,
                    name="es" if c < 2 else "eb",
                )
                for w0 in range(0, n_jt, WAVE):
                    jts = list(range(w0, min(w0 + WAVE, n_jt)))
                    sps = psS.tile([P, WAVE, CH], F32, name="sps")
                    for idx, jt in enumerate(jts):
                        nc.tensor.matmul(
                            sps[:, idx, :],
                            kT[h][:, ts(jt, P)],
                            qT[h][:, ts(c, CH)],
                            start=True,
                            stop=True,
                        )
                    for idx, jt in enumerate(jts):
                        b = jt - S_C * c
                        if b >= 0:
                            # mask the 128-wide diagonal block; cols < b*128
                            # are never read by PV, cols beyond are valid
                            nc.vector.tensor_add(
                                sps[:, idx, ts(b, P)], sps[:, idx, ts(b, P)],
                                dmask,
                            )
                    nc.scalar.activation(
                        expT[:, w0 : w0 + len(jts), :],
                        sps[:, : len(jts), :],
                        mybir.ActivationFunctionType.Exp,
                    )
                    fill()
                fill()
                for a in range(S_C):  # i-tile within chunk
                    i = S_C * c + a
                    z_ps = psZ.tile([P, E + 1], F32, name="z_ps")
                    for jt in range(i + 1):
                        nc.tensor.matmul(
                            z_ps,
                            expT[:, jt, ts(a, P)],
                            v_nat[:, jt, h, :],
                            start=(jt == 0),
                            stop=(jt == i),
                        )
                    rec = recp.tile([P, 1], F32, name="rec")
                    nc.vector.reciprocal(rec, z_ps[:, E : E + 1])
                    z_sb = zsb.tile([P, E], F16, name="z_sb")
                    nc.vector.tensor_scalar_mul(z_sb, z_ps[:, :E], rec)
                    tpz = psT.tile([P, P], F16, name="tpz")
                    nc.tensor.transpose(tpz, z_sb, identity)
                    nc.vector.tensor_copy(zT[:, h, ts(i, P)], tpz)

        # ---------------- master emission order ----------------
        # Prologue: v j-tiles 0..3 (need x chunk 0 only) + q/k for head 0.
        for jt in range(4):
            v_group(jt)
        for c in range(S_C):
            a_group(0, 0, c)
        for c in range(S_C):
            a_group(1, 0, c)

        # B(h0): fillers = remaining v groups + q/k(h1)
        f0 = [(lambda jt=jt: v_group(jt)) for jt in range(4, S_T)]
        f0 += [(lambda c=c: a_group(0, 1, c)) for c in range(S_C)]
        f0 += [(lambda c=c: a_group(1, 1, c)) for c in range(S_C)]
        b_head(0, f0)

        f1 = [(lambda c=c: a_group(0, 2, c)) for c in range(S_C)]
        f1 += [(lambda c=c: a_group(1, 2, c)) for c in range(S_C)]
        b_head(1, f1)

        f2 = [(lambda c=c: a_group(0, 3, c)) for c in range(S_C)]
        f2 += [(lambda c=c: a_group(1, 3, c)) for c in range(S_C)]
        b_head(2, f2)

        b_head(3, [])

        for t in range(S_T):
            for dc in range(D_C):
                c_group(t, dc)


_NC_CACHE = {}
LAST_RESULTS = None


def _get_nc():
    if "nc" not in _NC_CACHE:
        nc = bacc.Bacc("TRN2", target_bir_lowering=False, debug=False)
        xt = nc.dram_tensor("xt", [D, S], F16, kind="ExternalInput")
        wq = nc.dram_tensor("wq", [D, HL * E], F16, kind="ExternalInput")
        wk = nc.dram_tensor("wk", [D, HL * E], F16, kind="ExternalInput")
        wv = nc.dram_tensor("wv", [D, HL * E], F16, kind="ExternalInput")
        wo = nc.dram_tensor("wo", [HL * E, D], F16, kind="ExternalInput")
        bq = nc.dram_tensor("bq", [HL * E], F32, kind="ExternalInput")
        bk = nc.dram_tensor("bk", [HL * E], F32, kind="ExternalInput")
        outp = nc.dram_tensor("outp", [S, D], F16, kind="ExternalOutput")
        with tile.TileContext(nc) as tc:
            _trace_kernel(tc, xt, wq, wk, wv, wo, bq, bk, outp)
        nc.compile()
        _NC_CACHE["nc"] = nc
    return _NC_CACHE["nc"]


def kernel(normalized_resid_pre, W_Q, W_K, W_V, W_O, b_Q, b_K, b_V, b_O):
    x = np.asarray(normalized_resid_pre, np.float32)
    W_Q = np.asarray(W_Q, np.float32)
    W_K = np.asarray(W_K, np.float32)
    W_V = np.asarray(W_V, np.float32)
    W_O = np.asarray(W_O, np.float32)
    b_Q = np.asarray(b_Q, np.float32)
    b_K = np.asarray(b_K, np.float32)
    b_V = np.asarray(b_V, np.float32)
    b_O = np.asarray(b_O, np.float32)

    nc = _get_nc()
    in_maps = []
    for core in range(NCORES):
        b, g = core // (NCORES // B), core % (NCORES // B)
        hs = range(g * HL, (g + 1) * HL)
        in_maps.append(
            {
                "xt": np.ascontiguousarray(x[b].T).astype(np.float16),
                "wq": np.concatenate([W_Q[h] for h in hs], 1).astype(np.float16),
                "wk": np.concatenate([W_K[h] for h in hs], 1).astype(np.float16),
                "wv": np.concatenate([W_V[h] for h in hs], 1).astype(np.float16),
                "wo": W_O[g * HL : (g + 1) * HL].reshape(HL * E, D).astype(np.float16),
                "bq": np.ascontiguousarray(
                    b_Q[g * HL : (g + 1) * HL].reshape(-1) * np.float32(INV_SQRT_E)
                ),
                "bk": np.ascontiguousarray(b_K[g * HL : (g + 1) * HL].reshape(-1)),
            }
        )

    res = run_bass_kernel_spmd(nc, in_maps, core_ids=list(range(NCORES)))
    global LAST_RESULTS
    LAST_RESULTS = res
    out = np.zeros((B, S, D), np.float32)
    for core in range(NCORES):
        out[core // (NCORES // B)] += res.results[core]["outp"].astype(np.float32)
    # softmax rows sum to 1, so b_V contributes exactly b_V @ W_O per head
    out += (b_O + b_V.reshape(-1) @ W_O.reshape(H * E, D))[None, None, :]
    return out


# revision 5
# speedup vs baseline: 1.1429x; 1.1429x over previous
"""Multi-head causal attention on 8 Trainium2 NeuronCores.

Sharding: core c -> batch b = c // 4, head group g = c % 4 (4 of 16 heads).
Each core computes q/k/v for its 4 heads, causal softmax attention, and a
partial output  z_norm @ W_O[heads]  of shape [S, D].  Host sums the 4
head-group partials per batch and adds b_O (+ the exact b_V correction
sum_h b_V[h] @ W_O[h], since softmax rows sum to 1).

v3: bf16 datapath for all 512-free GEMM operands (measured: fp16 512-free
matmuls stream at 259 ns vs 227 ns for fp32r/bf16-class), fp16 for the
attention tensors (expT, v) where the 129-free PV matmuls run at full rate
and the extra mantissa helps softmax.  Everything SBUF-resident.  All big
matmul groups are emitted as PAIRS into the two 512-col halves of one
[128, 2, 512] PSUM slot with the contraction loop outermost, so
consecutive matmuls alternate PSUM banks (ILP) and the psum->SBUF copies
of one pair overlap the next pair's matmuls.  The projection pairs and
output-projection pairs are interleaved into the attention phase as PE
filler work, per chunk, so the PE never idles while ACT runs exp().

PSUM budget: pairs/waves 2x[128,2,512] + z 2x[128,129] + transpose
2x[128,128] = 8 banks.
"""

import sys

for _p in ("/opt/trn_rl_repo",):
    if _p not in sys.path:
        sys.path.insert(0, _p)

import numpy as np
import ml_dtypes

import concourse.bass as bass
from concourse import bacc
import concourse.mybir as mybir
import concourse.tile as tile
from concourse.bass_utils import run_bass_kernel_spmd
from concourse.masks import make_identity

F32 = mybir.dt.float32
BF16 = mybir.dt.bfloat16   # GEMM operands (512-free matmuls)
F16 = mybir.dt.float16     # attention operands (129-free PV matmuls)

B, S, D, H, E = 2, 2048, 2048, 16, 128
HL = 4          # heads per core
NCORES = 8
P = 128         # partitions
CH = 512        # free-dim chunk
S_T = S // P    # 16 seq tiles
S_C = S // CH   # 4 seq chunks
D_T = D // P    # 16 model-dim subtiles
D_C = D // CH   # 4 model-dim chunks
INV_SQRT_E = 1.0 / float(np.sqrt(E))
WAVE = 2        # j-tiles per scores wave (one PSUM pair-tile)


def _trace_kernel(tc, xt, wq, wk, wv, wo, bq, bk, outp):
    nc = tc.nc
    ts = bass.ts

    xt3 = xt.rearrange("(o p) s -> p o s", p=P)            # [128, 16, 2048]
    wq3 = wq.rearrange("(o p) e -> p o e", p=P)            # [128, 16, 512]
    wk3 = wk.rearrange("(o p) e -> p o e", p=P)
    wv3 = wv.rearrange("(o p) e -> p o e", p=P)
    wo3 = wo.rearrange("(h p) d -> p h d", p=P)            # [128, 4, 2048]
    out3 = outp.rearrange("(t p) d -> t p d", p=P)         # [16, 128, 2048]

    from contextlib import ExitStack

    with ExitStack() as top:
        const_pool = top.enter_context(tc.tile_pool(name="consts", bufs=1))
        xpool = top.enter_context(tc.tile_pool(name="x", bufs=1))
        wpool = top.enter_context(tc.tile_pool(name="w", bufs=1))
        wopool = top.enter_context(tc.tile_pool(name="wo", bufs=1))
        qkpool = top.enter_context(tc.tile_pool(name="qk", bufs=2))
        vpool = top.enter_context(tc.tile_pool(name="v", bufs=1))
        ztpool = top.enter_context(tc.tile_pool(name="zt", bufs=1))
        expp_s = top.enter_context(tc.tile_pool(name="exps", bufs=1))
        expp_b = top.enter_context(tc.tile_pool(name="expb", bufs=1))
        zsb = top.enter_context(tc.tile_pool(name="zsb", bufs=2))
        recp = top.enter_context(tc.tile_pool(name="rec", bufs=2))
        ostage = top.enter_context(tc.tile_pool(name="ost", bufs=3))
        psS = top.enter_context(tc.tile_pool(name="psS", bufs=2, space="PSUM"))
        psZ = top.enter_context(tc.tile_pool(name="psZ", bufs=2, space="PSUM"))
        psT = top.enter_context(tc.tile_pool(name="psT", bufs=2, space="PSUM"))

        # ---------------- constants ----------------
        identity_f = const_pool.tile([P, P], F32)
        make_identity(nc, identity_f)
        identity = const_pool.tile([P, P], BF16)
        nc.vector.tensor_copy(identity, identity_f)

        # transposed causal triangle for the diagonal 128-block of scoresT:
        # valid iff local col >= p  (j <= i)
        dmask = const_pool.tile([P, P], F32)
        nc.gpsimd.memset(dmask, 0.0)
        nc.gpsimd.affine_select(
            out=dmask,
            in_=dmask,
            compare_op=mybir.AluOpType.is_ge,
            fill=-30000.0,
            base=0,
            pattern=[[1, P]],
            channel_multiplier=-1,
        )

        biases = const_pool.tile([P, 2, HL], F32)
        nc.gpsimd.dma_start(biases[:, 0, :], bq.rearrange("(h p) -> p h", p=P))
        nc.gpsimd.dma_start(biases[:, 1, :], bk.rearrange("(h p) -> p h", p=P))

        # ---------------- input DMAs ----------------
        x_sb = xpool.tile([P, D_T, S], BF16)
        wv_sb = wpool.tile([P, D_T, HL * E], BF16, name="wv")
        wq_sb = wpool.tile([P, D_T, HL * E], BF16, name="wq")
        wk_sb = wpool.tile([P, D_T, HL * E], BF16, name="wk")
        wo_sb = wopool.tile([P, HL, D], BF16)

        for d in range(D_T):
            nc.sync.dma_start(wv_sb[:, d, :], wv3[:, d, :])
        for d in range(D_T):
            nc.sync.dma_start(x_sb[:, d, ts(0, CH)], xt3[:, d, ts(0, CH)])
        for d in range(D_T):
            nc.sync.dma_start(wq_sb[:, d, :], wq3[:, d, :])
        for d in range(D_T):
            nc.sync.dma_start(wk_sb[:, d, :], wk3[:, d, :])
        for c in range(1, S_C):
            for d in range(D_T):
                nc.sync.dma_start(x_sb[:, d, ts(c, CH)], xt3[:, d, ts(c, CH)])
        for lh in range(HL):
            for dc in range(D_C):
                nc.sync.dma_start(wo_sb[:, lh, ts(dc, CH)], wo3[:, lh, ts(dc, CH)])

        # v natural layout [j_in_tile, h, jt, e + ones-column], fp16
        v_nat = vpool.tile([P, HL, S_T, E + 1], F16)
        for lh in range(HL):
            nc.vector.memset(v_nat[:, lh, :, E : E + 1], 1.0)

        zT = ztpool.tile([P, HL, S], BF16)

        qT = {}
        kT = {}

        # ---------------- emission helpers ----------------
        def apair(h, c):
            """q AND k projection for (head h, seq chunk c), matmuls
            interleaved across the two halves (= two PSUM banks) of one
            pair slot; shared moving operand x[:, d, c]."""
            if c == 0:
                qT[h] = qkpool.tile([P, S], BF16, name="qT")
                kT[h] = qkpool.tile([P, S], BF16, name="kT")
            ps = psS.tile([P, 2, CH], F32, name="sps")
            for d in range(D_T):
                nc.tensor.matmul(
                    ps[:, 0, :],
                    wq_sb[:, d, ts(h, E)],
                    x_sb[:, d, ts(c, CH)],
                    start=(d == 0),
                    stop=(d == D_T - 1),
                )
                nc.tensor.matmul(
                    ps[:, 1, :],
                    wk_sb[:, d, ts(h, E)],
                    x_sb[:, d, ts(c, CH)],
                    start=(d == 0),
                    stop=(d == D_T - 1),
                )
            # q: bq is pre-scaled by 1/sqrt(E) on host
            nc.vector.tensor_scalar(
                qT[h][:, ts(c, CH)], ps[:, 0, :],
                INV_SQRT_E, biases[:, 0, h, None],
                op0=mybir.AluOpType.mult, op1=mybir.AluOpType.add,
            )
            nc.vector.tensor_scalar(
                kT[h][:, ts(c, CH)], ps[:, 1, :],
                1.0, biases[:, 1, h, None],
                op0=mybir.AluOpType.mult, op1=mybir.AluOpType.add,
            )

        def vpair(jt0, jt1):
            """v_nat for two j-tiles (all 4 heads each, free dim 512);
            stationary x slices, shared moving operand wv."""
            ps = psS.tile([P, 2, CH], F32, name="sps")
            for d in range(D_T):
                nc.tensor.matmul(
                    ps[:, 0, :],
                    x_sb[:, d, ts(jt0, P)],
                    wv_sb[:, d, :],
                    start=(d == 0),
                    stop=(d == D_T - 1),
                )
                nc.tensor.matmul(
                    ps[:, 1, :],
                    x_sb[:, d, ts(jt1, P)],
                    wv_sb[:, d, :],
                    start=(d == 0),
                    stop=(d == D_T - 1),
                )
            for g, jt in ((0, jt0), (1, jt1)):
                nc.vector.tensor_copy(
                    v_nat[:, :, jt, :E],
                    ps[:, g, :].rearrange("p (h e) -> p h e", h=HL),
                )

        def cpair(g0, g1):
            """Output projection for two (seq-tile, d-chunk) groups."""
            ps = psS.tile([P, 2, CH], F32, name="sps")
            for lh in range(HL):
                for g, (t, dc) in enumerate((g0, g1)):
                    nc.tensor.matmul(
                        ps[:, g, :],
                        zT[:, lh, ts(t, P)],
                        wo_sb[:, lh, ts(dc, CH)],
                        start=(lh == 0),
                        stop=(lh == HL - 1),
                    )
            for g, (t, dc) in enumerate((g0, g1)):
                ot = ostage.tile([P, CH], BF16, name="ot")
                nc.vector.tensor_copy(ot, ps[:, g, :])
                nc.sync.dma_start(out3[t, :, ts(dc, CH)], ot)

        def b_head(h, fillers_by_chunk):
            """Causal attention for one head; pops filler emissions (one
            per scores wave, rest drained before PV) to keep the PE busy
            while ACT runs exp()."""
            for c in range(S_C):
                fillers = fillers_by_chunk.get(c, [])
                n_jt = S_C * c + 4
                expT = (expp_s if c < 2 else expp_b).tile(
                    [P, 8 if c < 2 else 16, CH], F16,
                    name="es" if c < 2 else "eb",
                )
                for w0 in range(0, n_jt, WAVE):
                    jts = list(range(w0, min(w0 + WAVE, n_jt)))
                    sps = psS.tile([P, WAVE, CH], F32, name="sps")
                    for idx, jt in enumerate(jts):
                        nc.tensor.matmul(
                            sps[:, idx, :],
                            kT[h][:, ts(jt, P)],
                            qT[h][:, ts(c, CH)],
                            start=True,
                            stop=True,
                        )
                    for idx, jt in enumerate(jts):
                        b = jt - S_C * c
                        if b >= 0:
                            # mask the 128-wide diagonal block; cols < b*128
                            # are never read by PV, cols beyond are valid
                            nc.vector.tensor_add(
                                sps[:, idx, ts(b, P)], sps[:, idx, ts(b, P)],
                                dmask,
                            )
                    nc.scalar.activation(
                        expT[:, w0 : w0 + len(jts), :],
                        sps[:, : len(jts), :],
                        mybir.ActivationFunctionType.Exp,
                    )
                    if fillers:
                        fillers.pop(0)()
                for f in fillers:
                    f()
                for a in range(S_C):  # i-tile within chunk
                    i = S_C * c + a
                    z_ps = psZ.tile([P, E + 1], F32, name="z_ps")
                    for jt in range(i + 1):
                        nc.tensor.matmul(
                            z_ps,
                            expT[:, jt, ts(a, P)],
                            v_nat[:, h, jt, :],
                            start=(jt == 0),
                            stop=(jt == i),
                        )
                    rec = recp.tile([P, 1], F32, name="rec")
                    nc.vector.reciprocal(rec, z_ps[:, E : E + 1])
                    z_sb = zsb.tile([P, E], BF16, name="z_sb")
                    nc.vector.tensor_scalar_mul(z_sb, z_ps[:, :E], rec)
                    tpz = psT.tile([P, P], BF16, name="tpz")
                    nc.tensor.transpose(tpz, z_sb, identity)
                    nc.vector.tensor_copy(zT[:, h, ts(i, P)], tpz)

        # ---------------- master emission order ----------------
        vpair(0, 1)
        vpair(2, 3)
        for c in range(S_C):
            apair(0, c)

        b_head(0, {
            0: [lambda: vpair(4, 5)],
            1: [lambda: vpair(6, 7), lambda: apair(1, 0)],
            2: [lambda: vpair(8, 9), lambda: vpair(10, 11),
                lambda: apair(1, 1)],
            3: [lambda: vpair(12, 13), lambda: vpair(14, 15),
                lambda: apair(1, 2), lambda: apair(1, 3)],
        })
        b_head(1, {
            1: [lambda: apair(2, 0)],
            2: [lambda: apair(2, 1), lambda: apair(2, 2)],
            3: [lambda: apair(2, 3)],
        })
        b_head(2, {
            1: [lambda: apair(3, 0)],
            2: [lambda: apair(3, 1), lambda: apair(3, 2)],
            3: [lambda: apair(3, 3)],
        })
        # head 3 fillers: output projection for seq tiles t whose zT rows
        # are already complete across all heads (t < 4c during chunk c)
        cp = lambda t0, d0, t1, d1: (lambda: cpair((t0, d0), (t1, d1)))
        b_head(3, {
            1: [cp(0, 0, 0, 1), cp(0, 2, 0, 3), cp(1, 0, 1, 1), cp(1, 2, 1, 3)],
            2: [cp(2, 0, 2, 1), cp(2, 2, 2, 3), cp(3, 0, 3, 1), cp(3, 2, 3, 3)],
            3: [cp(4, 0, 4, 1), cp(4, 2, 4, 3), cp(5, 0, 5, 1), cp(5, 2, 5, 3),
                cp(6, 0, 6, 1), cp(6, 2, 6, 3), cp(7, 0, 7, 1), cp(7, 2, 7, 3)],
        })

        for t in range(8, S_T):
            for dc in range(0, D_C, 2):
                cpair((t, dc), (t, dc + 1))


_NC_CACHE = {}
LAST_RESULTS = None


def _get_nc():
    if "nc" not in _NC_CACHE:
        nc = bacc.Bacc("TRN2", target_bir_lowering=False, debug=False)
        xt = nc.dram_tensor("xt", [D, S], BF16, kind="ExternalInput")
        wq = nc.dram_tensor("wq", [D, HL * E], BF16, kind="ExternalInput")
        wk = nc.dram_tensor("wk", [D, HL * E], BF16, kind="ExternalInput")
        wv = nc.dram_tensor("wv", [D, HL * E], BF16, kind="ExternalInput")
        wo = nc.dram_tensor("wo", [HL * E, D], BF16, kind="ExternalInput")
        bq = nc.dram_tensor("bq", [HL * E], F32, kind="ExternalInput")
        bk = nc.dram_tensor("bk", [HL * E], F32, kind="ExternalInput")
        outp = nc.dram_tensor("outp", [S, D], BF16, kind="ExternalOutput")
        with tile.TileContext(nc) as tc:
            _trace_kernel(tc, xt, wq, wk, wv, wo, bq, bk, outp)
        nc.compile()
        _NC_CACHE["nc"] = nc
    return _NC_CACHE["nc"]


def kernel(normalized_resid_pre, W_Q, W_K, W_V, W_O, b_Q, b_K, b_V, b_O):
    x = np.asarray(normalized_resid_pre, np.float32)
    W_Q = np.asarray(W_Q, np.float32)
    W_K = np.asarray(W_K, np.float32)
    W_V = np.asarray(W_V, np.float32)
    W_O = np.asarray(W_O, np.float32)
    b_Q = np.asarray(b_Q, np.float32)
    b_K = np.asarray(b_K, np.float32)
    b_V = np.asarray(b_V, np.float32)
    b_O = np.asarray(b_O, np.float32)

    nc = _get_nc()
    bf16 = ml_dtypes.bfloat16
    in_maps = []
    for core in range(NCORES):
        b, g = core // (NCORES // B), core % (NCORES // B)
        hs = range(g * HL, (g + 1) * HL)
        in_maps.append(
            {
                "xt": np.ascontiguousarray(x[b].T).astype(bf16),
                "wq": np.concatenate([W_Q[h] for h in hs], 1).astype(bf16),
                "wk": np.concatenate([W_K[h] for h in hs], 1).astype(bf16),
                "wv": np.concatenate([W_V[h] for h in hs], 1).astype(bf16),
                "wo": W_O[g * HL : (g + 1) * HL].reshape(HL * E, D).astype(bf16),
                "bq": np.ascontiguousarray(
                    b_Q[g * HL : (g + 1) * HL].reshape(-1) * np.float32(INV_SQRT_E)
                ),
                "bk": np.ascontiguousarray(b_K[g * HL : (g + 1) * HL].reshape(-1)),
            }
        )

    res = run_bass_kernel_spmd(nc, in_maps, core_ids=list(range(NCORES)))
    global LAST_RESULTS
    LAST_RESULTS = res
    out = np.zeros((B, S, D), np.float32)
    for core in range(NCORES):
        out[core // (NCORES // B)] += np.asarray(
            res.results[core]["outp"]
        ).astype(np.float32)
    # softmax rows sum to 1, so b_V contributes exactly b_V @ W_O per head
    out += (b_O + b_V.reshape(-1) @ W_O.reshape(H * E, D))[None, None, :]
    return out


# revision 6
# speedup vs baseline: 1.2551x; 1.0981x over previous
"""Multi-head causal attention on 8 Trainium2 NeuronCores.

Sharding: core c -> batch b = c // 4, head group g = c % 4 (4 of 16 heads).
Each core computes q/k/v for its 4 heads, causal softmax attention, and a
partial output  z_norm @ W_O[heads]  of shape [S, D].  Host sums the 4
head-group partials per batch and adds b_O (+ the exact b_V correction
sum_h b_V[h] @ W_O[h], since softmax rows sum to 1).

v4: bf16 GEMM operands (512-free matmuls stream at 216 ns; fp16 measured
259 ns), fp16 attention operands (129-free PV matmuls run full rate).
Everything SBUF-resident.  The attention phase is ACT-limited at wave
granularity (exp of a 2-j-tile wave costs ~1.1 us vs 0.43 us of scores
matmuls), so all remaining projection / output-projection work is emitted
through GENERATORS that yield every 2 matmuls; after each wave's exp the
emitter ticks the generator queue a few steps, interleaving ~1.3 us of
independent PE work into each exp gap.  Diagonal-chunk scores matmuls are
trimmed to their valid column range (upper-left blocks are never read).

PSUM: waves 2x[128,2,512] + filler 1x[128,512] + z 2x[128,129] +
transpose 1x[128,128] = 8 banks.
"""

import sys

for _p in ("/opt/trn_rl_repo",):
    if _p not in sys.path:
        sys.path.insert(0, _p)

import numpy as np
import ml_dtypes

import concourse.bass as bass
from concourse import bacc
import concourse.mybir as mybir
import concourse.tile as tile
from concourse.bass_utils import run_bass_kernel_spmd
from concourse.masks import make_identity

F32 = mybir.dt.float32
BF16 = mybir.dt.bfloat16   # GEMM operands (512-free matmuls)
F16 = mybir.dt.float16     # attention operands (129-free PV matmuls)

B, S, D, H, E = 2, 2048, 2048, 16, 128
HL = 4          # heads per core
NCORES = 8
P = 128         # partitions
CH = 512        # free-dim chunk
S_T = S // P    # 16 seq tiles
S_C = S // CH   # 4 seq chunks
D_T = D // P    # 16 model-dim subtiles
D_C = D // CH   # 4 model-dim chunks
INV_SQRT_E = 1.0 / float(np.sqrt(E))
WAVE = 2        # j-tiles per scores wave (one PSUM pair-tile)


class Gen:
    """Resumable emission unit: advances one 'tick' (~2 matmuls) at a
    time so PE filler work can be interleaved at sub-microsecond grain."""

    def __init__(self, it):
        self.it = it
        self.done = False

    def step(self):
        if self.done:
            return False
        try:
            next(self.it)
            return True
        except StopIteration:
            self.done = True
            return False


def _trace_kernel(tc, xt, wq, wk, wv, wo, bq, bk, outp):
    nc = tc.nc
    ts = bass.ts

    xt3 = xt.rearrange("(o p) s -> p o s", p=P)            # [128, 16, 2048]
    wq3 = wq.rearrange("(o p) e -> p o e", p=P)            # [128, 16, 512]
    wk3 = wk.rearrange("(o p) e -> p o e", p=P)
    wv3 = wv.rearrange("(o p) e -> p o e", p=P)
    wo3 = wo.rearrange("(h p) d -> p h d", p=P)            # [128, 4, 2048]
    out3 = outp.rearrange("(t p) d -> t p d", p=P)         # [16, 128, 2048]

    from contextlib import ExitStack

    with ExitStack() as top:
        const_pool = top.enter_context(tc.tile_pool(name="consts", bufs=1))
        xpool = top.enter_context(tc.tile_pool(name="x", bufs=1))
        wpool = top.enter_context(tc.tile_pool(name="w", bufs=1))
        wopool = top.enter_context(tc.tile_pool(name="wo", bufs=1))
        qkpool = top.enter_context(tc.tile_pool(name="qk", bufs=2))
        vpool = top.enter_context(tc.tile_pool(name="v", bufs=1))
        ztpool = top.enter_context(tc.tile_pool(name="zt", bufs=1))
        expp_s = top.enter_context(tc.tile_pool(name="exps", bufs=1))
        expp_b = top.enter_context(tc.tile_pool(name="expb", bufs=1))
        zsb = top.enter_context(tc.tile_pool(name="zsb", bufs=2))
        recp = top.enter_context(tc.tile_pool(name="rec", bufs=2))
        ostage = top.enter_context(tc.tile_pool(name="ost", bufs=3))
        psS = top.enter_context(tc.tile_pool(name="psS", bufs=2, space="PSUM"))
        psF = top.enter_context(tc.tile_pool(name="psF", bufs=1, space="PSUM"))
        psZ = top.enter_context(tc.tile_pool(name="psZ", bufs=2, space="PSUM"))
        psT = top.enter_context(tc.tile_pool(name="psT", bufs=1, space="PSUM"))

        # ---------------- constants ----------------
        identity_f = const_pool.tile([P, P], F32)
        make_identity(nc, identity_f)
        identity = const_pool.tile([P, P], BF16)
        nc.vector.tensor_copy(identity, identity_f)

        # transposed causal triangle for the diagonal 128-block of scoresT:
        # valid iff local col >= p  (j <= i)
        dmask = const_pool.tile([P, P], F32)
        nc.gpsimd.memset(dmask, 0.0)
        nc.gpsimd.affine_select(
            out=dmask,
            in_=dmask,
            compare_op=mybir.AluOpType.is_ge,
            fill=-30000.0,
            base=0,
            pattern=[[1, P]],
            channel_multiplier=-1,
        )

        biases = const_pool.tile([P, 2, HL], F32)
        nc.gpsimd.dma_start(biases[:, 0, :], bq.rearrange("(h p) -> p h", p=P))
        nc.gpsimd.dma_start(biases[:, 1, :], bk.rearrange("(h p) -> p h", p=P))

        # ---------------- input DMAs ----------------
        x_sb = xpool.tile([P, D_T, S], BF16)
        wv_sb = wpool.tile([P, D_T, HL * E], BF16, name="wv")
        wq_sb = wpool.tile([P, D_T, HL * E], BF16, name="wq")
        wk_sb = wpool.tile([P, D_T, HL * E], BF16, name="wk")
        wo_sb = wopool.tile([P, HL, D], BF16)

        for d in range(D_T):
            nc.sync.dma_start(wv_sb[:, d, :], wv3[:, d, :])
        for d in range(D_T):
            nc.sync.dma_start(x_sb[:, d, ts(0, CH)], xt3[:, d, ts(0, CH)])
        for d in range(D_T):
            nc.sync.dma_start(wq_sb[:, d, :], wq3[:, d, :])
        for d in range(D_T):
            nc.sync.dma_start(wk_sb[:, d, :], wk3[:, d, :])
        for c in range(1, S_C):
            for d in range(D_T):
                nc.sync.dma_start(x_sb[:, d, ts(c, CH)], xt3[:, d, ts(c, CH)])
        for lh in range(HL):
            for dc in range(D_C):
                nc.sync.dma_start(wo_sb[:, lh, ts(dc, CH)], wo3[:, lh, ts(dc, CH)])

        # v natural layout [j_in_tile, h, jt, e + ones-column], fp16
        v_nat = vpool.tile([P, HL, S_T, E + 1], F16)
        for lh in range(HL):
            nc.vector.memset(v_nat[:, lh, :, E : E + 1], 1.0)

        zT = ztpool.tile([P, HL, S], BF16)

        qT = {}
        kT = {}

        # ---------------- prologue pair emitters (use wave slots) -------
        def apair(h, c):
            """q AND k projection for (head h, chunk c), matmuls
            interleaved across the two banks of one pair slot."""
            if c == 0:
                qT[h] = qkpool.tile([P, S], BF16, name="qT")
                kT[h] = qkpool.tile([P, S], BF16, name="kT")
            ps = psS.tile([P, 2, CH], F32, name="sps")
            for d in range(D_T):
                nc.tensor.matmul(
                    ps[:, 0, :], wq_sb[:, d, ts(h, E)], x_sb[:, d, ts(c, CH)],
                    start=(d == 0), stop=(d == D_T - 1),
                )
                nc.tensor.matmul(
                    ps[:, 1, :], wk_sb[:, d, ts(h, E)], x_sb[:, d, ts(c, CH)],
                    start=(d == 0), stop=(d == D_T - 1),
                )
            nc.vector.tensor_scalar(
                qT[h][:, ts(c, CH)], ps[:, 0, :],
                INV_SQRT_E, biases[:, 0, h, None],
                op0=mybir.AluOpType.mult, op1=mybir.AluOpType.add,
            )
            nc.vector.tensor_scalar(
                kT[h][:, ts(c, CH)], ps[:, 1, :],
                1.0, biases[:, 1, h, None],
                op0=mybir.AluOpType.mult, op1=mybir.AluOpType.add,
            )

        def vpair(jt0, jt1):
            ps = psS.tile([P, 2, CH], F32, name="sps")
            for d in range(D_T):
                nc.tensor.matmul(
                    ps[:, 0, :], x_sb[:, d, ts(jt0, P)], wv_sb[:, d, :],
                    start=(d == 0), stop=(d == D_T - 1),
                )
                nc.tensor.matmul(
                    ps[:, 1, :], x_sb[:, d, ts(jt1, P)], wv_sb[:, d, :],
                    start=(d == 0), stop=(d == D_T - 1),
                )
            for g, jt in ((0, jt0), (1, jt1)):
                nc.vector.tensor_copy(
                    v_nat[:, :, jt, :E],
                    ps[:, g, :].rearrange("p (h e) -> p h e", h=HL),
                )

        def cpair(g0, g1):
            ps = psS.tile([P, 2, CH], F32, name="sps")
            for lh in range(HL):
                for g, (t, dc) in enumerate((g0, g1)):
                    nc.tensor.matmul(
                        ps[:, g, :], zT[:, lh, ts(t, P)],
                        wo_sb[:, lh, ts(dc, CH)],
                        start=(lh == 0), stop=(lh == HL - 1),
                    )
            for g, (t, dc) in enumerate((g0, g1)):
                ot = ostage.tile([P, CH], BF16, name="ot")
                nc.vector.tensor_copy(ot, ps[:, g, :])
                nc.sync.dma_start(out3[t, :, ts(dc, CH)], ot)

        # ---------------- tick-granular filler generators ---------------
        def g_a(m, h, c):
            if c == 0 and m == 0:
                qT[h] = qkpool.tile([P, S], BF16, name="qT")
            if c == 0 and m == 1:
                kT[h] = qkpool.tile([P, S], BF16, name="kT")
            w_sb = wq_sb if m == 0 else wk_sb
            ps = psF.tile([P, CH], F32, name="fil")
            for d in range(D_T):
                nc.tensor.matmul(
                    ps, w_sb[:, d, ts(h, E)], x_sb[:, d, ts(c, CH)],
                    start=(d == 0), stop=(d == D_T - 1),
                )
                if d % 2 == 1 and d < D_T - 1:
                    yield
            nc.vector.tensor_scalar(
                (qT if m == 0 else kT)[h][:, ts(c, CH)], ps,
                INV_SQRT_E if m == 0 else 1.0,
                biases[:, m, h, None],
                op0=mybir.AluOpType.mult, op1=mybir.AluOpType.add,
            )

        def g_v(jt):
            ps = psF.tile([P, CH], F32, name="fil")
            for d in range(D_T):
                nc.tensor.matmul(
                    ps, x_sb[:, d, ts(jt, P)], wv_sb[:, d, :],
                    start=(d == 0), stop=(d == D_T - 1),
                )
                if d % 2 == 1 and d < D_T - 1:
                    yield
            nc.vector.tensor_copy(
                v_nat[:, :, jt, :E],
                ps.rearrange("p (h e) -> p h e", h=HL),
            )

        def g_c(t, dc):
            ps = psF.tile([P, CH], F32, name="fil")
            for lh in range(HL):
                nc.tensor.matmul(
                    ps, zT[:, lh, ts(t, P)], wo_sb[:, lh, ts(dc, CH)],
                    start=(lh == 0), stop=(lh == HL - 1),
                )
                if lh == 1:
                    yield
            ot = ostage.tile([P, CH], BF16, name="ot")
            nc.vector.tensor_copy(ot, ps)
            nc.sync.dma_start(out3[t, :, ts(dc, CH)], ot)

        queue = []

        def tick(n=1):
            for _ in range(n):
                while queue and not queue[0].step():
                    queue.pop(0)

        def require(gens):
            for g in gens:
                while not g.done:
                    tick()

        # ---------------- attention head emitter ------------------------
        def b_head(h, require_by_chunk=None, on_chunk=None):
            for c in range(S_C):
                if on_chunk:
                    on_chunk(c)
                n_jt = S_C * c + 4
                expT = (expp_s if c < 2 else expp_b).tile(
                    [P, 8 if c < 2 else 16, CH], F16,
                    name="es" if c < 2 else "eb",
                )
                for w0 in range(0, n_jt, WAVE):
                    jts = list(range(w0, min(w0 + WAVE, n_jt)))
                    sps = psS.tile([P, WAVE, CH], F32, name="sps")
                    for idx, jt in enumerate(jts):
                        b = jt - S_C * c
                        # diagonal-region j-tiles: columns < b*128 are
                        # never read downstream; skip computing them
                        o = b * P if b > 0 else 0
                        nc.tensor.matmul(
                            sps[:, idx, o:CH],
                            kT[h][:, ts(jt, P)],
                            qT[h][:, c * CH + o : (c + 1) * CH],
                            start=True,
                            stop=True,
                        )
                        if b >= 0:
                            nc.vector.tensor_add(
                                sps[:, idx, ts(b, P)], sps[:, idx, ts(b, P)],
                                dmask,
                            )
                    nc.scalar.activation(
                        expT[:, w0 : w0 + len(jts), :],
                        sps[:, : len(jts), :],
                        mybir.ActivationFunctionType.Exp,
                    )
                    tick(3)
                if require_by_chunk and c in require_by_chunk:
                    require(require_by_chunk[c])
                for a in range(S_C):  # i-tile within chunk
                    i = S_C * c + a
                    z_ps = psZ.tile([P, E + 1], F32, name="z_ps")
                    for jt in range(i + 1):
                        nc.tensor.matmul(
                            z_ps,
                            expT[:, jt, ts(a, P)],
                            v_nat[:, h, jt, :],
                            start=(jt == 0),
                            stop=(jt == i),
                        )
                    rec = recp.tile([P, 1], F32, name="rec")
                    nc.vector.reciprocal(rec, z_ps[:, E : E + 1])
                    z_sb = zsb.tile([P, E], BF16, name="z_sb")
                    nc.vector.tensor_scalar_mul(z_sb, z_ps[:, :E], rec)
                    tpz = psT.tile([P, P], BF16, name="tpz")
                    nc.tensor.transpose(tpz, z_sb, identity)
                    nc.vector.tensor_copy(zT[:, h, ts(i, P)], tpz)

        # ---------------- master emission order ----------------
        vpair(0, 1)
        vpair(2, 3)
        for c in range(S_C):
            apair(0, c)

        gv = {jt: Gen(g_v(jt)) for jt in range(4, S_T)}
        ga = {(m, h, c): Gen(g_a(m, h, c))
              for h in (1, 2, 3) for m in (0, 1) for c in range(S_C)}
        queue.extend(gv[jt] for jt in range(4, S_T))
        for h in (1, 2, 3):
            for m in (0, 1):
                for c in range(S_C):
                    queue.append(ga[(m, h, c)])

        b_head(0, require_by_chunk={
            1: [gv[jt] for jt in range(4, 8)],
            2: [gv[jt] for jt in range(8, 12)],
            3: [gv[jt] for jt in range(12, 16)],
        })
        require([ga[(m, 1, c)] for m in (0, 1) for c in range(S_C)])
        b_head(1)
        require([ga[(m, 2, c)] for m in (0, 1) for c in range(S_C)])
        b_head(2)
        require([ga[(m, 3, c)] for m in (0, 1) for c in range(S_C)])

        # head 3: stream output-projection groups into the queue per chunk,
        # gated to seq tiles whose zT rows are complete across all heads
        def h3_chunk(c):
            if c == 0:
                return
            ts_lo = {1: 0, 2: 2, 3: 4}[c]
            ts_hi = {1: 2, 2: 4, 3: 8}[c]
            for t in range(ts_lo, ts_hi):
                for dc in range(D_C):
                    queue.append(Gen(g_c(t, dc)))

        b_head(3, on_chunk=h3_chunk)
        while queue:
            tick()

        for t in range(8, S_T):
            for dc in range(0, D_C, 2):
                cpair((t, dc), (t, dc + 1))


_NC_CACHE = {}
LAST_RESULTS = None


def _get_nc():
    if "nc" not in _NC_CACHE:
        nc = bacc.Bacc("TRN2", target_bir_lowering=False, debug=False)
        xt = nc.dram_tensor("xt", [D, S], BF16, kind="ExternalInput")
        wq = nc.dram_tensor("wq", [D, HL * E], BF16, kind="ExternalInput")
        wk = nc.dram_tensor("wk", [D, HL * E], BF16, kind="ExternalInput")
        wv = nc.dram_tensor("wv", [D, HL * E], BF16, kind="ExternalInput")
        wo = nc.dram_tensor("wo", [HL * E, D], BF16, kind="ExternalInput")
        bq = nc.dram_tensor("bq", [HL * E], F32, kind="ExternalInput")
        bk = nc.dram_tensor("bk", [HL * E], F32, kind="ExternalInput")
        outp = nc.dram_tensor("outp", [S, D], BF16, kind="ExternalOutput")
        with tile.TileContext(nc) as tc:
            _trace_kernel(tc, xt, wq, wk, wv, wo, bq, bk, outp)
        nc.compile()
        _NC_CACHE["nc"] = nc
    return _NC_CACHE["nc"]


def kernel(normalized_resid_pre, W_Q, W_K, W_V, W_O, b_Q, b_K, b_V, b_O):
    x = np.asarray(normalized_resid_pre, np.float32)
    W_Q = np.asarray(W_Q, np.float32)
    W_K = np.asarray(W_K, np.float32)
    W_V = np.asarray(W_V, np.float32)
    W_O = np.asarray(W_O, np.float32)
    b_Q = np.asarray(b_Q, np.float32)
    b_K = np.asarray(b_K, np.float32)
    b_V = np.asarray(b_V, np.float32)
    b_O = np.asarray(b_O, np.float32)

    nc = _get_nc()
    bf16 = ml_dtypes.bfloat16
    in_maps = []
    for core in range(NCORES):
        b, g = core // (NCORES // B), core % (NCORES // B)
        hs = range(g * HL, (g + 1) * HL)
        in_maps.append(
            {
                "xt": np.ascontiguousarray(x[b].T).astype(bf16),
                "wq": np.concatenate([W_Q[h] for h in hs], 1).astype(bf16),
                "wk": np.concatenate([W_K[h] for h in hs], 1).astype(bf16),
                "wv": np.concatenate([W_V[h] for h in hs], 1).astype(bf16),
                "wo": W_O[g * HL : (g + 1) * HL].reshape(HL * E, D).astype(bf16),
                "bq": np.ascontiguousarray(
                    b_Q[g * HL : (g + 1) * HL].reshape(-1) * np.float32(INV_SQRT_E)
                ),
                "bk": np.ascontiguousarray(b_K[g * HL : (g + 1) * HL].reshape(-1)),
            }
        )

    res = run_bass_kernel_spmd(nc, in_maps, core_ids=list(range(NCORES)))
    global LAST_RESULTS
    LAST_RESULTS = res
    out = np.zeros((B, S, D), np.float32)
    for core in range(NCORES):
        out[core // (NCORES // B)] += np.asarray(
            res.results[core]["outp"]
        ).astype(np.float32)
    # softmax rows sum to 1, so b_V contributes exactly b_V @ W_O per head
    out += (b_O + b_V.reshape(-1) @ W_O.reshape(H * E, D))[None, None, :]
    return out
